# revision 28
# baseline (speedup 1.0000x reference)
"""YOLOv4-style detection loss on 8 Trainium2 NeuronCores.

Strategy (pure data parallel, 2 images per core; the 6 scalar losses are
summed on the host, the degenerate all-reduce for scalars):

  Sparsity: of the 85 channels only channel 4 (objectness) contributes to
  the loss at every cell. The other 84 channels matter only at the <=100
  label-assigned target cells per image, plus channels 0-3 wherever a
  small label could trigger the IoU>0.5 ignore test. That ignore set is
  provably confined to a tiny window around each small-enough label
  (larger labels can never reach IoU 0.5 against the ~1x1 pred boxes),
  so it is evaluated exactly on the host as a sparse correction, the
  same way the per-target constants and anchor matching are host label
  math (per the data-parallel sharding hint).

  Host prep per core: label math (anchor CIoU argmax replicated in f32,
  target-cell dedup with last-write-wins, per-target constants), packing
  the <=100 target cells' 85-channel rows plus constants into one small
  [NT, 338] tensor (one contiguous DMA instead of shipping an 11.8MB
  transposed copy of x to feed a 68KB indirect gather), and the flat
  padded [128, 136]-per-image channel-4 plane.

  Device (Bass/Tile, one program SPMD on 8 cores) does all O(A*F*F)
  dense work and all per-target-cell tensor math:
  - dense channel 4: exp/ln chains on [128, 272] (full 128-partition
    utilization; engine cost scales with free size only) giving
    sum(softplus(v4)) and sum(sigmoid(v4)^2) per image;
  - per-target bce/l2 partials via fused-both-image ACT sigmoid chains
    and short DVE accumulation chains; everything lands in a [128, 20]
    partials tile DMA'd out raw (no on-device reduction matmul).

  Host combines the 8 cores' [128, 20] partials with the host-known
  per-target weights (w2, 0.5*w2, m) into the 6 outputs.
"""

import numpy as np
from contextlib import ExitStack

N_CLASSES = 80
N_ANCHORS = 3
IMAGE_SIZE = 608
STRIDE = 8
FSIZE = 76
BATCH = 16
N_BOX = 100
N_CH = 85
NCELL = FSIZE * FSIZE  # 5776
N_CORES = 8
IMG_PER_CORE = BATCH // N_CORES  # 2
PCOL = 136                       # 17328 cells padded to 128*136
NPAD = 128 * PCOL - N_ANCHORS * NCELL  # 80 zero-pad cells per image
TGW = IMG_PER_CORE * (N_CH + 4)        # target rows + small constants (178)
NOHW = IMG_PER_CORE * N_CLASSES        # one-cold class masks (160)

ANCHORS_PX = np.array([[13, 16], [28, 32], [62, 35]], dtype=np.float32)
MA = ANCHORS_PX / IMAGE_SIZE / STRIDE  # [3,2] f32, grid-normalized

LN2 = float(np.log(np.float32(2.0)))


# ----------------------------------------------------------------- host prep

def _best_n(lw, lh):
    """Replicates reference _iou_xyxy_ciou((0,0,lw,lh), (0,0,aw,ah)) argmax in f32."""
    f32 = np.float32
    ious = np.zeros((lw.shape[0], 3), np.float32)
    coef = f32(4.0 / np.pi**2)
    for k in range(3):
        aw, ah = f32(MA[k, 0]), f32(MA[k, 1])
        brx = np.minimum(lw, aw)
        bry = np.minimum(lh, ah)
        area_a = lw * lh
        area_b = aw * ah
        en = ((brx > 0) & (bry > 0)).astype(np.float32)
        ai = brx * bry * en
        iou = ai / np.maximum(area_a + area_b - ai, f32(1e-16))
        rho2 = (lw / 2 - aw / 2) ** 2 + (lh / 2 - ah / 2) ** 2
        c2 = lw**2 + lh**2
        v = coef * (np.arctan(lw / np.maximum(lh, f32(1e-16)))
                    - f32(np.arctan(aw / max(ah, f32(1e-16))))) ** 2
        alpha = v / np.maximum(1 - iou + v, f32(1e-16))
        ious[:, k] = iou - rho2 / np.maximum(c2, f32(1e-16)) - alpha * v
    return np.argmax(ious, axis=1).astype(np.int32)


def _sigmoid32(v):
    return (1.0 / (1.0 + np.exp(-v.astype(np.float32)))).astype(np.float32)


def _ignore_correction(xb, lx, ly, lw, lh, small_idx, tgt_flat):
    """Exact obj/l2 dense correction for ignored (IoU>0.5) non-target cells.

    xb: [3, 85, 5776] one image of x. Returns (d_obj, d_l2): the sums of
    softplus(v4) and sigmoid(v4)^2 over ignored non-target cells. Only
    cells inside the provable reach window of each small label can be
    ignored, so this is O(#small * window) work.
    """
    f32 = np.float32
    d_obj = 0.0
    d_l2 = 0.0
    if len(small_idx) == 0:
        return d_obj, d_l2
    counted = set()
    for a in range(N_ANCHORS):
        # per-anchor bound on pred box extents
        pwmax = float(np.exp(np.abs(xb[a, 2]).max() * MA[a, 0]) * (1 + 1e-5))
        phmax = float(np.exp(np.abs(xb[a, 3]).max() * MA[a, 1]) * (1 + 1e-5))
        for s in small_idx:
            lxm = f32(lx[s] - lw[s] * f32(0.5))
            lxM = f32(lx[s] + lw[s] * f32(0.5))
            lym = f32(ly[s] - lh[s] * f32(0.5))
            lyM = f32(ly[s] + lh[s] * f32(0.5))
            al = f32(lw[s] * lh[s])
            i0 = max(0, int(np.floor(lxm - pwmax / 2)) - 1)
            i1 = min(FSIZE - 1, int(np.ceil(lxM + pwmax / 2)) + 1)
            j0 = max(0, int(np.floor(lym - phmax / 2)) - 1)
            j1 = min(FSIZE - 1, int(np.ceil(lyM + phmax / 2)) + 1)
            if i1 < i0 or j1 < j0:
                continue
            ii = np.arange(i0, i1 + 1, dtype=np.int32)
            jj = np.arange(j0, j1 + 1, dtype=np.int32)
            cell = (jj[:, None] * FSIZE + ii[None, :]).ravel()
            v0 = xb[a, 0, cell]; v1 = xb[a, 1, cell]
            v2 = xb[a, 2, cell]; v3 = xb[a, 3, cell]
            v4 = xb[a, 4, cell]
            px = _sigmoid32(v0) + np.tile(ii, len(jj)).astype(np.float32)
            py = _sigmoid32(v1) + np.repeat(jj, len(ii)).astype(np.float32)
            pw = np.exp(v2 * f32(MA[a, 0])).astype(np.float32)
            ph = np.exp(v3 * f32(MA[a, 1])).astype(np.float32)
            ap = pw * ph
            iw = (np.minimum(px + pw * f32(0.5), lxM)
                  - np.maximum(px - pw * f32(0.5), lxm))
            ih = (np.minimum(py + ph * f32(0.5), lyM)
                  - np.maximum(py - ph * f32(0.5), lym))
            ai = np.maximum(iw, 0) * np.maximum(ih, 0)
            ig = (f32(3.0) * ai - ap) > al
            for k in np.nonzero(ig)[0]:
                flat = a * NCELL + int(cell[k])
                if flat in counted or flat in tgt_flat:
                    continue
                counted.add(flat)
                v = np.float64(v4[k])
                d_obj += float(np.log1p(np.exp(v)))
                d_l2 += float(1.0 / (1.0 + np.exp(-v))) ** 2
    return d_obj, d_l2


def prep_inputs(x, labels):
    """Host-side label math. Returns per-core input maps + host-side state."""
    f32 = np.float32
    x = np.ascontiguousarray(x, dtype=np.float32)
    labels = np.asarray(labels, dtype=np.float32)

    lx = (labels[:, :, 0] + labels[:, :, 2]) / f32(STRIDE * 2)
    ly = (labels[:, :, 1] + labels[:, :, 3]) / f32(STRIDE * 2)
    lw = labels[:, :, 2] / f32(STRIDE)
    lh = labels[:, :, 3] / f32(STRIDE)
    li = lx.astype(np.int32)
    lj = ly.astype(np.int32)

    # conservative bound on pred box area: only labels with grid area below
    # 2*max(pred area) can ever reach IoU > 0.5 (3*ai > ap+al with ai <= ap)
    xr = x.reshape(BATCH, N_ANCHORS, N_CH, NCELL)
    apmax = 0.0
    for a in range(3):
        m2 = float(np.abs(xr[:, a, 2]).max())
        m3 = float(np.abs(xr[:, a, 3]).max())
        apmax = max(apmax, float(np.exp(m2 * MA[a, 0]) * np.exp(m3 * MA[a, 1])))
    small_thr = f32(2.0 * apmax * (1.0 + 1e-4))
    small_mask = (lw * lh) < small_thr  # [B, N_BOX]

    percore = []
    NT = 1
    for c in range(N_CORES):
        bs = [c * IMG_PER_CORE + i for i in range(IMG_PER_CORE)]
        xc4 = np.zeros((128, IMG_PER_CORE * PCOL), np.float32)
        himg = []
        pimg = []
        for bi, b in enumerate(bs):
            xb = xr[b]  # [3, 85, 5776]
            # flat channel-4 plane, cell c at (c % 128, c // 128), zero pad
            v4flat = np.zeros(128 * PCOL, np.float32)
            v4flat[:N_ANCHORS * NCELL] = xb[:, 4, :].reshape(-1)
            xc4[:, bi * PCOL:(bi + 1) * PCOL] = v4flat.reshape(PCOL, 128).T

            bn = _best_n(lw[b], lh[b])
            cell = lj[b] * FSIZE + li[b]
            flat = bn * NCELL + cell
            # last write wins (XLA CPU scatter semantics for duplicate indices)
            win = {}
            for t in range(N_BOX):
                win[int(flat[t])] = t
            ts = sorted(win.values())
            n = len(ts)
            NT = max(NT, n)
            idx = np.array(ts, np.int32)
            a_t = bn[idx]
            c_t = cell[idx]
            aw = MA[a_t, 0].astype(np.float32)
            ah = MA[a_t, 1].astype(np.float32)
            tx = lx[b, idx] - np.trunc(lx[b, idx])
            tw = np.log(lw[b, idx] / aw + f32(1e-16))
            th = np.log(lh[b, idx] / ah + f32(1e-16))
            scale_v = np.sqrt(f32(2.0) - lw[b, idx] * lh[b, idx]
                              / f32(NCELL * 1.0))
            w2 = (scale_v * scale_v).astype(np.float32)
            # the 85-channel rows of the n target cells
            rows = xb[a_t[:, None], np.arange(N_CH)[None, :], c_t[:, None]]
            tcc = np.zeros((n, 84), np.float32)
            tcc[:, 0] = f32(1.0) - tx
            tcc[:, 1] = tw
            tcc[:, 2] = th
            tcc[:, 3] = tx
            cls = labels[b, idx, 4].astype(np.int32)
            noh = np.ones((n, N_CLASSES), np.float32)
            noh[np.arange(n), cls] = 0.0
            tcc[:, 4:84] = noh
            pimg.append((n, rows.astype(np.float32), tcc))

            # exact sparse ignore correction (non-target cells only)
            tgt_flat = set(int(v) for v in (a_t * NCELL + c_t))
            sidx = np.nonzero(small_mask[b])[0]
            d_obj, d_l2 = _ignore_correction(
                xb, lx[b], ly[b], lw[b], lh[b], sidx, tgt_flat)
            himg.append({'n': n, 'w2': w2, 'd_obj': d_obj, 'd_l2': d_l2})
        percore.append((xc4, pimg, himg))

    in_maps = []
    host = []
    for xc4, pimg, himg in percore:
        tgtc = np.zeros((NT, TGW), np.float32)
        noh = np.zeros((NT, NOHW), np.float32)
        for bi, (n, rows, tcc) in enumerate(pimg):
            tgtc[:n, bi * N_CH:(bi + 1) * N_CH] = rows
            co = IMG_PER_CORE * N_CH + bi * 4
            tgtc[:n, co:co + 4] = tcc[:, 0:4]
            noh[:n, bi * N_CLASSES:(bi + 1) * N_CLASSES] = tcc[:, 4:84]
        in_maps.append({"xc4": np.ascontiguousarray(xc4),
                        "tgtc": tgtc, "noh": noh})
        host.append(himg)
    return in_maps, host, NT


# ----------------------------------------------------------------- device IR

def _pin_act_table():
    """All activations here use exp/ln, which coexist in the
    natural_log_exp_and_others table. The default table chooser ping-pongs
    between single-function tables (~1.3us per load); empty out every other
    set (names and positions preserved so act_func_set ids stay valid) so
    exactly one table load is emitted."""
    import concourse.bacc as bacc
    import concourse.hw_specs as hw_specs
    if getattr(bacc, "_act_tbl_pinned", False):
        return
    orig = hw_specs.get_activation_tables
    keep = "natural_log_exp_and_others"

    def pinned(arch):
        t = orig(arch)
        return {name: (fns if name == keep else set())
                for name, fns in t.items()}

    bacc.get_activation_tables = pinned
    bacc._act_tbl_pinned = True


def build_program(NT):
    import concourse.bacc as bacc
    import concourse.tile as tile
    from concourse.tile import add_dep_helper
    from concourse import mybir

    _pin_act_table()

    f32 = mybir.dt.float32
    AF = mybir.ActivationFunctionType
    OP = mybir.AluOpType
    NP = 20  # parts columns
    T2 = IMG_PER_CORE * N_CH  # 170

    nc = bacc.Bacc("TRN2", target_bir_lowering=False, debug=False)
    xc4_t = nc.dram_tensor("xc4", [128, IMG_PER_CORE * PCOL], f32,
                           kind="ExternalInput")
    tgtc_t = nc.dram_tensor("tgtc", [NT, TGW], f32, kind="ExternalInput")
    noh_t = nc.dram_tensor("noh", [NT, NOHW], f32, kind="ExternalInput")
    out_t = nc.dram_tensor("out", [128, NP], f32, kind="ExternalOutput")

    with tile.TileContext(nc) as tcx, ExitStack() as ctx:
        sb = ctx.enter_context(tcx.tile_pool(name="sb", bufs=2))
        acc = ctx.enter_context(tcx.tile_pool(name="acc", bufs=1))

        # ---- loads (HWDGE, latency-priority order): the narrow target-row
        # block gates the long sigmoid chain; the wide one-cold class masks
        # are only needed by mid-timeline DVE ops
        TGTC = acc.tile([NT, TGW], f32)
        nc.sync.dma_start(TGTC[:], tgtc_t.ap())
        XC4 = acc.tile([128, IMG_PER_CORE * PCOL], f32)
        nc.sync.dma_start(XC4[:], xc4_t.ap())
        NOHT = acc.tile([NT, NOHW], f32)
        nc.sync.dma_start(NOHT[:], noh_t.ap())

        parts = acc.tile([128, NP], f32)
        nc.gpsimd.memset(parts[:], 0.0)

        TG = TGTC[0:NT, 0:T2]

        # ---- ACT stream, ordered so each op's input is >=2 ops back
        # (hides the ~220ns same-engine dependency latency):
        #   E, L, E4, S, L4, SQ4
        E = acc.tile([NT, T2], f32)
        nc.scalar.activation(E[:], TG, AF.Exp, scale=-1.0)
        L = acc.tile([NT, T2], f32)
        li = nc.scalar.activation(L[:], E[:], AF.Ln, bias=1.0)
        E4 = acc.tile([128, IMG_PER_CORE * PCOL], f32)
        e4i = nc.scalar.activation(E4[:], XC4[:], AF.Exp, scale=-1.0)
        add_dep_helper(e4i.ins, li.ins, sync=False,
                       reason="order: dense exp after target ln")
        S = acc.tile([NT, T2], f32)
        si = nc.scalar.activation(S[:], L[:], AF.Exp, scale=-1.0)
        add_dep_helper(si.ins, e4i.ins, sync=False,
                       reason="order: target sigmoid after dense exp")
        # dense ln/exp per image so each half's reduction starts earlier
        L4 = acc.tile([128, IMG_PER_CORE * PCOL], f32)
        SQ4 = acc.tile([128, IMG_PER_CORE * PCOL], f32)
        prev = si
        l4i = [None] * IMG_PER_CORE
        sq4i = [None] * IMG_PER_CORE
        for img in range(IMG_PER_CORE):
            sl = slice(img * PCOL, (img + 1) * PCOL)
            l4i[img] = nc.scalar.activation(L4[:, sl], E4[:, sl], AF.Ln,
                                            bias=1.0)
            add_dep_helper(l4i[img].ins, prev.ins, sync=False,
                           reason="order: dense ln placement")
            prev = l4i[img]
        for img in range(IMG_PER_CORE):
            sl = slice(img * PCOL, (img + 1) * PCOL)
            # the last image's sigmoid^2 sum rides the ACT accumulator (the
            # DVE stream is saturated by then; ACT finishes it sooner)
            accum = (parts[:, 2 + img:3 + img]
                     if img == IMG_PER_CORE - 1 else None)
            sq4i[img] = nc.scalar.activation(SQ4[:, sl], L4[:, sl], AF.Exp,
                                             scale=-2.0, accum_out=accum)
            add_dep_helper(sq4i[img].ins, prev.ins, sync=False,
                           reason="order: dense exp placement")
            prev = sq4i[img]

        def pair_reduce(dst, src):
            return nc.vector.tensor_reduce(
                dst, src.rearrange("p (i c) -> p i c", i=IMG_PER_CORE),
                axis=mybir.AxisListType.X, op=OP.add)

        def half_reduce(dst, src, img):
            sl = slice(img * PCOL, (img + 1) * PCOL)
            return nc.vector.tensor_reduce(dst, src[:, sl],
                                           axis=mybir.AxisListType.X,
                                           op=OP.add)

        # ---- per-target partials, emitted in dependency phases so the DVE
        # stream never head-of-line blocks on late producers
        def img_views(img):
            co = 6 + img * 7
            o = img * N_CH
            tco = T2 + img * 4
            col = lambda j: TGTC[0:NT, tco + j:tco + j + 1]
            NOH = NOHT[0:NT, img * N_CLASSES:(img + 1) * N_CLASSES]
            return (co, TGTC[0:NT, o:o + N_CH], L[0:NT, o:o + N_CH],
                    S[0:NT, o:o + N_CH], col, NOH, tco)

        # phase 1: raw-target-row consumers
        a01s, Us = [], []
        for img in range(IMG_PER_CORE):
            co, TGi, Li, Si, col, NOH, tco = img_views(img)
            a01 = sb.tile([NT, 1], f32, tag="a01", name=f"a01_{img}")
            nc.gpsimd.tensor_tensor(a01[:], TGi[:, 0:1], TGi[:, 1:2],
                                    op=OP.add)
            a01s.append(a01)
            U = sb.tile([NT, 2], f32, tag="U", name=f"U_{img}")
            nc.gpsimd.tensor_tensor(U[:], TGi[:, 2:4],
                                    TGTC[0:NT, tco + 1:tco + 3],
                                    op=OP.subtract)
            Us.append(U)
            # obj target correction: -v4 (dense pass counted softplus(v4))
            nc.gpsimd.tensor_scalar_mul(parts[0:NT, co + 3:co + 4],
                                        TGi[:, 4:5], -1.0)
            # wh: sum (v23 - twh)^2  (host applies 0.5*w2 / w2)
            U2 = sb.tile([NT, 2], f32, tag="U2")
            nc.vector.scalar_tensor_tensor(U2[:], U[:], 1.0, U[:],
                                           OP.mult, OP.mult,
                                           accum_out=parts[0:NT,
                                                           co + 2:co + 3])

        pair_reduce(parts[:, 4:6], XC4[:])       # sum v4 per image

        # phase 2: L-dependent
        for img in range(IMG_PER_CORE):
            co, TGi, Li, Si, col, NOH, tco = img_views(img)
            b01 = sb.tile([NT, 1], f32, tag="b01", name=f"b01_{img}")
            nc.gpsimd.tensor_tensor(b01[:], Li[:, 0:1], Li[:, 1:2],
                                    op=OP.add)
            # cls bce part B: sum_c L_c
            nc.vector.tensor_reduce(parts[0:NT, co + 5:co + 6],
                                    Li[:, 5:N_CH],
                                    axis=mybir.AxisListType.X, op=OP.add)
            # xy bce: (1-tx)*(v0+v1) + (L0+L1)
            nc.vector.scalar_tensor_tensor(parts[0:NT, co + 0:co + 1],
                                           a01s[img][:], col(0), b01[:],
                                           OP.mult, OP.add)

        # cls bce part A (needs the late-arriving one-cold masks)
        for img in range(IMG_PER_CORE):
            co, TGi, Li, Si, col, NOH, tco = img_views(img)
            JA = sb.tile([NT, N_CLASSES], f32, tag="JA")
            nc.vector.scalar_tensor_tensor(JA[:], TGi[:, 5:N_CH], 1.0, NOH,
                                           OP.mult, OP.mult,
                                           accum_out=parts[0:NT,
                                                           co + 4:co + 5])

        # phase 3: S-dependent chains; per-image dense reductions interleaved
        # so each starts as soon as its producer half finishes
        D01s = []
        for img in range(IMG_PER_CORE):
            co, TGi, Li, Si, col, NOH, tco = img_views(img)
            D01 = sb.tile([NT, 2], f32, tag="D01", name=f"D01_{img}")
            nc.gpsimd.tensor_single_scalar(D01[:], Si[:, 0:2], col(3),
                                           OP.subtract)
            D01s.append(D01)
        for img in range(IMG_PER_CORE):
            co, TGi, Li, Si, col, NOH, tco = img_views(img)
            if img == IMG_PER_CORE - 1:
                # image 0's sigmoid^2 sum is ready by now; slot it in ahead
                # of the final cls-l2 block so it doesn't trail the stream
                half_reduce(parts[:, 2:3], SQ4, 0)
            # xy l2: sum (sigmoid - tx)^2  (obj l2 target term folded below)
            D01b = sb.tile([NT, 2], f32, tag="D01b")
            l2xy = sb.tile([NT, 1], f32, tag="l2xy", name=f"l2xy_{img}")
            nc.vector.scalar_tensor_tensor(D01b[:], D01s[img][:], 1.0,
                                           D01s[img][:], OP.mult, OP.mult,
                                           accum_out=l2xy[:])
            # cls l2: sum ((S_c - 1) + noh_c)^2
            Dc = sb.tile([NT, N_CLASSES], f32, tag="Dc")
            nc.vector.scalar_tensor_tensor(Dc[:], Si[:, 5:N_CH], -1.0, NOH,
                                           OP.add, OP.add)
            Dc2 = sb.tile([NT, N_CLASSES], f32, tag="Dc2")
            nc.vector.scalar_tensor_tensor(Dc2[:], Dc[:], 1.0, Dc[:],
                                           OP.mult, OP.mult,
                                           accum_out=parts[0:NT,
                                                           co + 6:co + 7])
            # obj l2 target correction -2*S4 folds into the xy-l2 column:
            # host reads col as l2xy - 2*S4 (plus +1 per row added on host)
            nc.vector.scalar_tensor_tensor(parts[0:NT, co + 1:co + 2],
                                           Si[:, 4:5], -2.0, l2xy[:],
                                           OP.mult, OP.add)
            # softplus sum for this image (its dense half is finished)
            half_reduce(parts[:, 0 + img:1 + img], L4, img)

        nc.sync.dma_start(out_t.ap(), parts[:])

    nc.compile()
    return nc


_prog_cache = {}


def _get_program(nt):
    if nt not in _prog_cache:
        _prog_cache[nt] = build_program(nt)
    return _prog_cache[nt]


def kernel(x, labels):
    from concourse.bass_utils import run_bass_kernel_spmd

    in_maps, host, NT = prep_inputs(np.asarray(x), np.asarray(labels))
    nc = _get_program(NT)
    res = run_bass_kernel_spmd(nc, in_maps, list(range(N_CORES)))

    xy = wh = obj = cls = l2 = 0.0
    for c in range(N_CORES):
        o = np.asarray(res.results[c]["out"], np.float64)
        for img in range(IMG_PER_CORE):
            h = host[c][img]
            n = h['n']
            w2 = h['w2'].astype(np.float64)
            co = 6 + img * 7
            # dense channel-4 partials (partition-summed), minus zero-pad
            obj += o[:, 4 + img].sum() + o[:, 0 + img].sum() - NPAD * LN2 \
                - h['d_obj']
            l2 += o[:, 2 + img].sum() - NPAD * 0.25 - h['d_l2']
            # per-target partials
            xy += (w2[:n] * o[:n, co + 0]).sum()
            whss = o[:n, co + 2]
            wh += (0.5 * w2[:n] * whss).sum()
            l2 += (w2[:n] * whss).sum()
            obj += o[:n, co + 3].sum()
            cls += o[:n, co + 4].sum() + o[:n, co + 5].sum()
            l2 += o[:n, co + 1].sum() + o[:n, co + 6].sum() + n
    loss = xy + wh + obj + cls
    return np.array([loss, xy, wh, obj, cls, l2], np.float32)


# revision 38
# speedup vs baseline: 1.0324x; 1.0324x over previous
"""YOLOv4-style detection loss on 8 Trainium2 NeuronCores.

Strategy (pure data parallel, 2 images per core; the 6 scalar losses are
summed on the host, the degenerate all-reduce for scalars):

  Sparsity: of the 85 channels only channel 4 (objectness) contributes to
  the loss at every cell. The other 84 channels matter only at the <=100
  label-assigned target cells per image, plus channels 0-3 wherever a
  small label could trigger the IoU>0.5 ignore test. That ignore set is
  provably confined to a tiny window around each small-enough label
  (larger labels can never reach IoU 0.5 against the ~1x1 pred boxes),
  so it is evaluated exactly on the host as a sparse correction, the
  same way the per-target constants and anchor matching are host label
  math (per the data-parallel sharding hint).

  Host prep per core: label math (anchor CIoU argmax replicated in f32,
  target-cell dedup with last-write-wins, per-target constants), packing
  the <=100 target cells' 85-channel rows plus constants into one small
  [NT, 338] tensor (one contiguous DMA instead of shipping an 11.8MB
  transposed copy of x to feed a 68KB indirect gather), and the flat
  padded [128, 136]-per-image channel-4 plane.

  Device (Bass/Tile, one program SPMD on 8 cores) does all O(A*F*F)
  dense work and all per-target-cell tensor math:
  - dense channel 4: exp/ln chains on [128, 272] (full 128-partition
    utilization; engine cost scales with free size only) giving
    sum(softplus(v4)) and sum(sigmoid(v4)^2) per image;
  - per-target bce/l2 partials via fused-both-image ACT sigmoid chains
    and short DVE accumulation chains; everything lands in a [128, 20]
    partials tile DMA'd out raw (no on-device reduction matmul).

  Host combines the 8 cores' [128, 20] partials with the host-known
  per-target weights (w2, 0.5*w2, m) into the 6 outputs.
"""

import numpy as np
from contextlib import ExitStack

N_CLASSES = 80
N_ANCHORS = 3
IMAGE_SIZE = 608
STRIDE = 8
FSIZE = 76
BATCH = 16
N_BOX = 100
N_CH = 85
NCELL = FSIZE * FSIZE  # 5776
N_CORES = 8
IMG_PER_CORE = BATCH // N_CORES  # 2
PCOL = 136                       # 17328 cells padded to 128*136
NPAD = 128 * PCOL - N_ANCHORS * NCELL  # 80 zero-pad cells per image
TGW = IMG_PER_CORE * (N_CH + 4)        # target rows + small constants (178)

ANCHORS_PX = np.array([[13, 16], [28, 32], [62, 35]], dtype=np.float32)
MA = ANCHORS_PX / IMAGE_SIZE / STRIDE  # [3,2] f32, grid-normalized

LN2 = float(np.log(np.float32(2.0)))


# ----------------------------------------------------------------- host prep

def _best_n(lw, lh):
    """Replicates reference _iou_xyxy_ciou((0,0,lw,lh), (0,0,aw,ah)) argmax in f32."""
    f32 = np.float32
    ious = np.zeros((lw.shape[0], 3), np.float32)
    coef = f32(4.0 / np.pi**2)
    for k in range(3):
        aw, ah = f32(MA[k, 0]), f32(MA[k, 1])
        brx = np.minimum(lw, aw)
        bry = np.minimum(lh, ah)
        area_a = lw * lh
        area_b = aw * ah
        en = ((brx > 0) & (bry > 0)).astype(np.float32)
        ai = brx * bry * en
        iou = ai / np.maximum(area_a + area_b - ai, f32(1e-16))
        rho2 = (lw / 2 - aw / 2) ** 2 + (lh / 2 - ah / 2) ** 2
        c2 = lw**2 + lh**2
        v = coef * (np.arctan(lw / np.maximum(lh, f32(1e-16)))
                    - f32(np.arctan(aw / max(ah, f32(1e-16))))) ** 2
        alpha = v / np.maximum(1 - iou + v, f32(1e-16))
        ious[:, k] = iou - rho2 / np.maximum(c2, f32(1e-16)) - alpha * v
    return np.argmax(ious, axis=1).astype(np.int32)


def _sigmoid32(v):
    return (1.0 / (1.0 + np.exp(-v.astype(np.float32)))).astype(np.float32)


def _ignore_correction(xb, lx, ly, lw, lh, small_idx, tgt_flat):
    """Exact obj/l2 dense correction for ignored (IoU>0.5) non-target cells.

    xb: [3, 85, 5776] one image of x. Returns (d_obj, d_l2): the sums of
    softplus(v4) and sigmoid(v4)^2 over ignored non-target cells. Only
    cells inside the provable reach window of each small label can be
    ignored, so this is O(#small * window) work.
    """
    f32 = np.float32
    d_obj = 0.0
    d_l2 = 0.0
    if len(small_idx) == 0:
        return d_obj, d_l2
    counted = set()
    for a in range(N_ANCHORS):
        # per-anchor bound on pred box extents
        pwmax = float(np.exp(np.abs(xb[a, 2]).max() * MA[a, 0]) * (1 + 1e-5))
        phmax = float(np.exp(np.abs(xb[a, 3]).max() * MA[a, 1]) * (1 + 1e-5))
        for s in small_idx:
            lxm = f32(lx[s] - lw[s] * f32(0.5))
            lxM = f32(lx[s] + lw[s] * f32(0.5))
            lym = f32(ly[s] - lh[s] * f32(0.5))
            lyM = f32(ly[s] + lh[s] * f32(0.5))
            al = f32(lw[s] * lh[s])
            i0 = max(0, int(np.floor(lxm - pwmax / 2)) - 1)
            i1 = min(FSIZE - 1, int(np.ceil(lxM + pwmax / 2)) + 1)
            j0 = max(0, int(np.floor(lym - phmax / 2)) - 1)
            j1 = min(FSIZE - 1, int(np.ceil(lyM + phmax / 2)) + 1)
            if i1 < i0 or j1 < j0:
                continue
            ii = np.arange(i0, i1 + 1, dtype=np.int32)
            jj = np.arange(j0, j1 + 1, dtype=np.int32)
            cell = (jj[:, None] * FSIZE + ii[None, :]).ravel()
            v0 = xb[a, 0, cell]; v1 = xb[a, 1, cell]
            v2 = xb[a, 2, cell]; v3 = xb[a, 3, cell]
            v4 = xb[a, 4, cell]
            px = _sigmoid32(v0) + np.tile(ii, len(jj)).astype(np.float32)
            py = _sigmoid32(v1) + np.repeat(jj, len(ii)).astype(np.float32)
            pw = np.exp(v2 * f32(MA[a, 0])).astype(np.float32)
            ph = np.exp(v3 * f32(MA[a, 1])).astype(np.float32)
            ap = pw * ph
            iw = (np.minimum(px + pw * f32(0.5), lxM)
                  - np.maximum(px - pw * f32(0.5), lxm))
            ih = (np.minimum(py + ph * f32(0.5), lyM)
                  - np.maximum(py - ph * f32(0.5), lym))
            ai = np.maximum(iw, 0) * np.maximum(ih, 0)
            ig = (f32(3.0) * ai - ap) > al
            for k in np.nonzero(ig)[0]:
                flat = a * NCELL + int(cell[k])
                if flat in counted or flat in tgt_flat:
                    continue
                counted.add(flat)
                v = np.float64(v4[k])
                d_obj += float(np.log1p(np.exp(v)))
                d_l2 += float(1.0 / (1.0 + np.exp(-v))) ** 2
    return d_obj, d_l2


def prep_inputs(x, labels):
    """Host-side label math. Returns per-core input maps + host-side state."""
    f32 = np.float32
    x = np.ascontiguousarray(x, dtype=np.float32)
    labels = np.asarray(labels, dtype=np.float32)

    lx = (labels[:, :, 0] + labels[:, :, 2]) / f32(STRIDE * 2)
    ly = (labels[:, :, 1] + labels[:, :, 3]) / f32(STRIDE * 2)
    lw = labels[:, :, 2] / f32(STRIDE)
    lh = labels[:, :, 3] / f32(STRIDE)
    li = lx.astype(np.int32)
    lj = ly.astype(np.int32)

    # conservative bound on pred box area: only labels with grid area below
    # 2*max(pred area) can ever reach IoU > 0.5 (3*ai > ap+al with ai <= ap)
    xr = x.reshape(BATCH, N_ANCHORS, N_CH, NCELL)
    apmax = 0.0
    for a in range(3):
        m2 = float(np.abs(xr[:, a, 2]).max())
        m3 = float(np.abs(xr[:, a, 3]).max())
        apmax = max(apmax, float(np.exp(m2 * MA[a, 0]) * np.exp(m3 * MA[a, 1])))
    small_thr = f32(2.0 * apmax * (1.0 + 1e-4))
    small_mask = (lw * lh) < small_thr  # [B, N_BOX]

    percore = []
    NT = 1
    for c in range(N_CORES):
        bs = [c * IMG_PER_CORE + i for i in range(IMG_PER_CORE)]
        xc4 = np.zeros((128, IMG_PER_CORE * PCOL), np.float32)
        himg = []
        pimg = []
        for bi, b in enumerate(bs):
            xb = xr[b]  # [3, 85, 5776]
            # flat channel-4 plane, cell c at (c % 128, c // 128), zero pad
            v4flat = np.zeros(128 * PCOL, np.float32)
            v4flat[:N_ANCHORS * NCELL] = xb[:, 4, :].reshape(-1)
            xc4[:, bi * PCOL:(bi + 1) * PCOL] = v4flat.reshape(PCOL, 128).T

            bn = _best_n(lw[b], lh[b])
            cell = lj[b] * FSIZE + li[b]
            flat = bn * NCELL + cell
            # last write wins (XLA CPU scatter semantics for duplicate indices)
            win = {}
            for t in range(N_BOX):
                win[int(flat[t])] = t
            ts = sorted(win.values())
            n = len(ts)
            NT = max(NT, n)
            idx = np.array(ts, np.int32)
            a_t = bn[idx]
            c_t = cell[idx]
            aw = MA[a_t, 0].astype(np.float32)
            ah = MA[a_t, 1].astype(np.float32)
            tx = lx[b, idx] - np.trunc(lx[b, idx])
            tw = np.log(lw[b, idx] / aw + f32(1e-16))
            th = np.log(lh[b, idx] / ah + f32(1e-16))
            scale_v = np.sqrt(f32(2.0) - lw[b, idx] * lh[b, idx]
                              / f32(NCELL * 1.0))
            w2 = (scale_v * scale_v).astype(np.float32)
            # the 85-channel rows of the n target cells
            rows = xb[a_t[:, None], np.arange(N_CH)[None, :], c_t[:, None]]
            rows = rows.astype(np.float32)
            tcc = np.zeros((n, 4), np.float32)
            tcc[:, 0] = f32(1.0) - tx
            tcc[:, 1] = tw
            tcc[:, 2] = th
            tcc[:, 3] = tx
            # per-target class-column values: the device computes class sums
            # over all 80 classes; the host re-adds the one-hot column terms
            cls = labels[b, idx, 4].astype(np.int32)
            vcls = rows[np.arange(n), 5 + cls]
            scls = _sigmoid32(vcls)
            pimg.append((n, rows, tcc))

            # exact sparse ignore correction (non-target cells only)
            tgt_flat = set(int(v) for v in (a_t * NCELL + c_t))
            sidx = np.nonzero(small_mask[b])[0]
            d_obj, d_l2 = _ignore_correction(
                xb, lx[b], ly[b], lw[b], lh[b], sidx, tgt_flat)
            himg.append({'n': n, 'w2': w2, 'd_obj': d_obj, 'd_l2': d_l2,
                         'vcls': float(vcls.astype(np.float64).sum()),
                         'scls': float(scls.astype(np.float64).sum())})
        percore.append((xc4, pimg, himg))

    in_maps = []
    host = []
    for xc4, pimg, himg in percore:
        tgtc = np.zeros((NT, TGW), np.float32)
        for bi, (n, rows, tcc) in enumerate(pimg):
            tgtc[:n, bi * N_CH:(bi + 1) * N_CH] = rows
            co = IMG_PER_CORE * N_CH + bi * 4
            tgtc[:n, co:co + 4] = tcc
        in_maps.append({"xc4": np.ascontiguousarray(xc4), "tgtc": tgtc})
        host.append(himg)
    return in_maps, host, NT


# ----------------------------------------------------------------- device IR

def _pin_act_table():
    """All activations here use exp/ln, which coexist in the
    natural_log_exp_and_others table. The default table chooser ping-pongs
    between single-function tables (~1.3us per load); empty out every other
    set (names and positions preserved so act_func_set ids stay valid) so
    exactly one table load is emitted."""
    import concourse.bacc as bacc
    import concourse.hw_specs as hw_specs
    if getattr(bacc, "_act_tbl_pinned", False):
        return
    orig = hw_specs.get_activation_tables
    keep = "natural_log_exp_and_others"

    def pinned(arch):
        t = orig(arch)
        return {name: (fns if name == keep else set())
                for name, fns in t.items()}

    bacc.get_activation_tables = pinned
    bacc._act_tbl_pinned = True


def build_program(NT):
    import concourse.bacc as bacc
    import concourse.tile as tile
    from concourse.tile import add_dep_helper
    from concourse import mybir

    _pin_act_table()

    f32 = mybir.dt.float32
    AF = mybir.ActivationFunctionType
    OP = mybir.AluOpType
    NP = 20  # parts columns
    T2 = IMG_PER_CORE * N_CH  # 170

    nc = bacc.Bacc("TRN2", target_bir_lowering=False, debug=False)
    xc4_t = nc.dram_tensor("xc4", [128, IMG_PER_CORE * PCOL], f32,
                           kind="ExternalInput")
    tgtc_t = nc.dram_tensor("tgtc", [NT, TGW], f32, kind="ExternalInput")
    out_t = nc.dram_tensor("out", [128, NP], f32, kind="ExternalOutput")

    with tile.TileContext(nc) as tcx, ExitStack() as ctx:
        sb = ctx.enter_context(tcx.tile_pool(name="sb", bufs=2))
        acc = ctx.enter_context(tcx.tile_pool(name="acc", bufs=1))

        # ---- loads (HWDGE, latency-priority order): the narrow target-row
        # block gates the long sigmoid chain; the wide one-cold class masks
        # are only needed by mid-timeline DVE ops
        TGTC = acc.tile([NT, TGW], f32)
        nc.sync.dma_start(TGTC[:], tgtc_t.ap())
        XC4 = acc.tile([128, IMG_PER_CORE * PCOL], f32)
        nc.sync.dma_start(XC4[:], xc4_t.ap())

        parts = acc.tile([128, NP], f32)
        nc.gpsimd.memset(parts[:], 0.0)

        TG = TGTC[0:NT, 0:T2]

        # ---- ACT stream, ordered so each op's input is >=2 ops back
        # (hides the ~220ns same-engine dependency latency):
        #   E, L, E4, S, L4, SQ4
        E = acc.tile([NT, T2], f32)
        nc.scalar.activation(E[:], TG, AF.Exp, scale=-1.0)
        L = acc.tile([NT, T2], f32)
        li = nc.scalar.activation(L[:], E[:], AF.Ln, bias=1.0)
        E4 = acc.tile([128, IMG_PER_CORE * PCOL], f32)
        e4i = nc.scalar.activation(E4[:], XC4[:], AF.Exp, scale=-1.0)
        add_dep_helper(e4i.ins, li.ins, sync=False,
                       reason="order: dense exp after target ln")
        S = acc.tile([NT, T2], f32)
        si = nc.scalar.activation(S[:], L[:], AF.Exp, scale=-1.0)
        add_dep_helper(si.ins, e4i.ins, sync=False,
                       reason="order: target sigmoid after dense exp")
        # dense ln/exp per image so each half's reduction starts earlier
        L4 = acc.tile([128, IMG_PER_CORE * PCOL], f32)
        SQ4 = acc.tile([128, IMG_PER_CORE * PCOL], f32)
        prev = si
        l4i = [None] * IMG_PER_CORE
        sq4i = [None] * IMG_PER_CORE
        for img in range(IMG_PER_CORE):
            sl = slice(img * PCOL, (img + 1) * PCOL)
            l4i[img] = nc.scalar.activation(L4[:, sl], E4[:, sl], AF.Ln,
                                            bias=1.0)
            add_dep_helper(l4i[img].ins, prev.ins, sync=False,
                           reason="order: dense ln placement")
            prev = l4i[img]
        for img in range(IMG_PER_CORE):
            sl = slice(img * PCOL, (img + 1) * PCOL)
            # the last image's sigmoid^2 sum rides the ACT accumulator (the
            # DVE stream is saturated by then; ACT finishes it sooner)
            accum = (parts[:, 2 + img:3 + img]
                     if img == IMG_PER_CORE - 1 else None)
            sq4i[img] = nc.scalar.activation(SQ4[:, sl], L4[:, sl], AF.Exp,
                                             scale=-2.0, accum_out=accum)
            add_dep_helper(sq4i[img].ins, prev.ins, sync=False,
                           reason="order: dense exp placement")
            prev = sq4i[img]

        def pair_reduce(dst, src):
            return nc.vector.tensor_reduce(
                dst, src.rearrange("p (i c) -> p i c", i=IMG_PER_CORE),
                axis=mybir.AxisListType.X, op=OP.add)

        def half_reduce(dst, src, img):
            sl = slice(img * PCOL, (img + 1) * PCOL)
            return nc.vector.tensor_reduce(dst, src[:, sl],
                                           axis=mybir.AxisListType.X,
                                           op=OP.add)

        # ---- per-target partials, emitted in dependency phases so the DVE
        # stream never head-of-line blocks on late producers
        def img_views(img):
            co = 6 + img * 7
            o = img * N_CH
            tco = T2 + img * 4
            col = lambda j: TGTC[0:NT, tco + j:tco + j + 1]
            return (co, TGTC[0:NT, o:o + N_CH], L[0:NT, o:o + N_CH],
                    S[0:NT, o:o + N_CH], col, tco)

        # phase 1: raw-target-row consumers
        a01s = []
        for img in range(IMG_PER_CORE):
            co, TGi, Li, Si, col, tco = img_views(img)
            a01 = sb.tile([NT, 1], f32, tag="a01", name=f"a01_{img}")
            nc.gpsimd.tensor_tensor(a01[:], TGi[:, 0:1], TGi[:, 1:2],
                                    op=OP.add)
            a01s.append(a01)
            U = sb.tile([NT, 2], f32, tag="U", name=f"U_{img}")
            nc.gpsimd.tensor_tensor(U[:], TGi[:, 2:4],
                                    TGTC[0:NT, tco + 1:tco + 3],
                                    op=OP.subtract)
            # obj target correction: -v4 (dense pass counted softplus(v4))
            nc.gpsimd.tensor_scalar_mul(parts[0:NT, co + 3:co + 4],
                                        TGi[:, 4:5], -1.0)
            # wh: sum (v23 - twh)^2  (host applies 0.5*w2 / w2)
            U2 = sb.tile([NT, 2], f32, tag="U2")
            nc.vector.scalar_tensor_tensor(U2[:], U[:], 1.0, U[:],
                                           OP.mult, OP.mult,
                                           accum_out=parts[0:NT,
                                                           co + 2:co + 3])
            # cls bce raw part: sum_c v_c (host subtracts the one-hot column)
            nc.vector.tensor_reduce(parts[0:NT, co + 4:co + 5],
                                    TGi[:, 5:N_CH],
                                    axis=mybir.AxisListType.X, op=OP.add)

        pair_reduce(parts[:, 4:6], XC4[:])       # sum v4 per image

        # phase 2: L-dependent
        for img in range(IMG_PER_CORE):
            co, TGi, Li, Si, col, tco = img_views(img)
            b01 = sb.tile([NT, 1], f32, tag="b01", name=f"b01_{img}")
            nc.gpsimd.tensor_tensor(b01[:], Li[:, 0:1], Li[:, 1:2],
                                    op=OP.add)
            # cls bce ln part: sum_c L_c
            nc.vector.tensor_reduce(parts[0:NT, co + 5:co + 6],
                                    Li[:, 5:N_CH],
                                    axis=mybir.AxisListType.X, op=OP.add)
            # xy bce: (1-tx)*(v0+v1) + (L0+L1)
            nc.vector.scalar_tensor_tensor(parts[0:NT, co + 0:co + 1],
                                           a01s[img][:], col(0), b01[:],
                                           OP.mult, OP.add)

        # phase 3: S-dependent chains; per-image dense reductions interleaved
        # so each starts as soon as its producer half finishes
        for img in range(IMG_PER_CORE):
            co, TGi, Li, Si, col, tco = img_views(img)
            if img == IMG_PER_CORE - 1:
                # image 0's sigmoid^2 sum is ready by now; slot it in ahead
                # of the final blocks so it doesn't trail the stream
                half_reduce(parts[:, 2:3], SQ4, 0)
            # xy l2: sum (sigmoid - tx)^2  (obj l2 target term folded below)
            D01 = sb.tile([NT, 2], f32, tag="D01", name=f"D01_{img}")
            nc.vector.tensor_single_scalar(D01[:], Si[:, 0:2], col(3),
                                           OP.subtract)
            D01b = sb.tile([NT, 2], f32, tag="D01b")
            l2xy = sb.tile([NT, 1], f32, tag="l2xy", name=f"l2xy_{img}")
            nc.vector.scalar_tensor_tensor(D01b[:], D01[:], 1.0, D01[:],
                                           OP.mult, OP.mult,
                                           accum_out=l2xy[:])
            # cls l2 square part: sum_c S_c^2 (host adds 1 - 2*sigma_cls)
            SQc = sb.tile([NT, N_CLASSES], f32, tag="SQc")
            nc.vector.scalar_tensor_tensor(
                SQc[:], Si[:, 5:N_CH], 1.0, Si[:, 5:N_CH], OP.mult, OP.mult,
                accum_out=parts[0:NT, co + 6:co + 7])
            # obj l2 target correction -2*S4 folds into the xy-l2 column:
            # host reads col as l2xy - 2*S4 (plus +1 per row added on host)
            nc.vector.scalar_tensor_tensor(parts[0:NT, co + 1:co + 2],
                                           Si[:, 4:5], -2.0, l2xy[:],
                                           OP.mult, OP.add)
            # softplus sum for this image (its dense half is finished)
            half_reduce(parts[:, 0 + img:1 + img], L4, img)

        nc.sync.dma_start(out_t.ap(), parts[:])

    nc.compile()
    return nc


_prog_cache = {}


def _get_program(nt):
    if nt not in _prog_cache:
        _prog_cache[nt] = build_program(nt)
    return _prog_cache[nt]


def kernel(x, labels):
    from concourse.bass_utils import run_bass_kernel_spmd

    in_maps, host, NT = prep_inputs(np.asarray(x), np.asarray(labels))
    nc = _get_program(NT)
    res = run_bass_kernel_spmd(nc, in_maps, list(range(N_CORES)))

    xy = wh = obj = cls = l2 = 0.0
    for c in range(N_CORES):
        o = np.asarray(res.results[c]["out"], np.float64)
        for img in range(IMG_PER_CORE):
            h = host[c][img]
            n = h['n']
            w2 = h['w2'].astype(np.float64)
            co = 6 + img * 7
            # dense channel-4 partials (partition-summed), minus zero-pad
            obj += o[:, 4 + img].sum() + o[:, 0 + img].sum() - NPAD * LN2 \
                - h['d_obj']
            l2 += o[:, 2 + img].sum() - NPAD * 0.25 - h['d_l2']
            # per-target partials
            xy += (w2[:n] * o[:n, co + 0]).sum()
            whss = o[:n, co + 2]
            wh += (0.5 * w2[:n] * whss).sum()
            l2 += (w2[:n] * whss).sum()
            obj += o[:n, co + 3].sum()
            cls += o[:n, co + 4].sum() + o[:n, co + 5].sum() - h['vcls']
            l2 += (o[:n, co + 1].sum() + o[:n, co + 6].sum() + 2 * n
                   - 2.0 * h['scls'])
    loss = xy + wh + obj + cls
    return np.array([loss, xy, wh, obj, cls, l2], np.float32)
